# revision 18
# baseline (speedup 1.0000x reference)
"""Trainium2 Bass kernel for nn_DiscreteWaveletTransform (3-level db4 DWT,
symmetric padding, + linear resize of each coefficient band back to T).

Approach: the whole per-signal pipeline (3 DWT levels + 4 resizes) is one
fixed linear operator out[t, c] = sum_k sig[k] * M[k, 4t+c].  M (2048 x 8192)
is banded: the 512 output columns of one 128-wide t-chunk depend on <= 194
consecutive k rows, and the band start advances by exactly 128 k per t-chunk.
Blocking k on a grid shifted by SHIFT (so every t-chunk's band falls in two
consecutive 128-row blocks) gives exactly 2 PE matmuls per t-chunk:
[K=128, M=128 signals, N<=512 cols] in bf16, accumulated in PSUM.  Within
each 128-row block only a contiguous column range is nonzero, so each matmul
streams just that range (~71% of 2x512 total) and packed M stores only those
ranges; PSUM has_written semantics stitch the two ranges together.  The matmul
orientation puts signals on PSUM partitions and (t, c)-interleaved columns on
the free axis, so each PSUM tile is a contiguous [128 signals, 128 t x 4 c]
chunk of the output layout — no transposes, contiguous output DMAs.

The output is stored to DRAM in bf16 (halves the dominant HBM stream) and
upcast to float32 on the host during the gather.

Sharding: data-parallel over B (16 -> 2 per core); each core handles
2 x 512 = 1024 signals = 8 blocks of 128.
"""

from contextlib import ExitStack

import numpy as np

import concourse.bacc as bacc
import concourse.bass as bass
import concourse.tile as tile
from concourse import mybir
from concourse.bass_utils import run_bass_kernel_spmd

# ---------------------------------------------------------------- problem dims
import os  # noqa: E402

B, T, N = 16, 2048, 512
LEVELS = 3
C = LEVELS + 1
F = 8
NCORES = 8
B_PER_CORE = B // NCORES          # 2
TCHUNKS = T // 128                # 16
SBLOCKS = B_PER_CORE * (N // 128)  # 8 signal blocks of 128 per core
OGROUP = int(os.environ.get("K_OGROUP", "8"))  # t-chunks per output DMA
OUT_RING = os.environ.get("K_OUT_RING", "alt")   # act | alt  (output DMA queue)
MPREFETCH = os.environ.get("K_MPREFETCH", "jit")  # jit | top  (M load placement)
COPY_SPLIT = int(os.environ.get("K_COPY_SPLIT", "2"))  # n-1 of n copies on DVE
OBUFS = int(os.environ.get("K_OBUFS", "0"))      # opool bufs override (0=auto)
MCOMPRESS = int(os.environ.get("K_MCOMPRESS", "0"))  # DMA/matmul only nonzero M rows
SIGSPLIT = int(os.environ.get("K_SIGSPLIT", "0"))    # split first sig load
PROBE = os.environ.get("K_PROBE", "full")  # full | dma | pe  (timing probes)
STAGGER = int(os.environ.get("K_STAGGER", "1"))  # staggered sem reset in For_i
MGROUP = int(os.environ.get("K_MGROUP", "2"))    # M pairs per load DMA (2|8|32)
SIGPAIR = int(os.environ.get("K_SIGPAIR", "0"))  # 2 signal blocks per sig DMA
SBUFS = int(os.environ.get("K_SBUFS", "3"))      # spool bufs (sig prefetch depth)
MBUFS = int(os.environ.get("K_MBUFS", "1"))      # mpool bufs (cross-rep M decouple)
SHIFT = 60                         # k-grid shift aligning blocks to the band
NBLK = (T + SHIFT + 127) // 128    # 17 shifted k-blocks cover [-SHIFT, 2116)

DEC_LO = np.array([-0.010597401784997278, 0.032883011666982945, 0.030841381835986965,
                   -0.18703481171888114, -0.02798376941698385, 0.6308807679295904,
                   0.7148465705525415, 0.23037781330885523])
DEC_HI = np.array([-0.23037781330885523, 0.7148465705525415, -0.6308807679295904,
                   -0.02798376941698385, 0.18703481171888114, 0.030841381835986965,
                   -0.032883011666982945, -0.010597401784997278])


# ------------------------------------------------------- operator construction
def _dwt_step(sig, lo, hi):
    S = sig.shape[1]
    ext = np.pad(sig, ((0, 0), (F - 1, F - 1)), mode='symmetric')[:, 1:]
    L = (S + F - 1) // 2
    lo_r, hi_r = lo[::-1], hi[::-1]
    cA = sum(ext[:, k:k + 2 * L:2] * lo_r[k] for k in range(F))
    cD = sum(ext[:, k:k + 2 * L:2] * hi_r[k] for k in range(F))
    return cA, cD


def _resize(c, t):
    S = c.shape[-1]
    if S == t:
        return c
    if S > t:
        return c[..., :t]
    pos = (np.arange(t, dtype=c.dtype) + 0.5) * (S / t) - 0.5
    pos = np.clip(pos, 0.0, S - 1)
    lo = np.floor(pos).astype(np.int64)
    hi = np.minimum(lo + 1, S - 1)
    w = pos - lo.astype(c.dtype)
    return c[..., lo] * (1.0 - w) + c[..., hi] * w


def _build_operator():
    """M [T, T, C] float64: out[s, t, c] = sum_k sig[s, k] M[k, t, c]."""
    a = np.eye(T)
    details = []
    for _ in range(LEVELS):
        a, d = _dwt_step(a, DEC_LO, DEC_HI)
        details.append(d)
    coeffs = [a] + details[::-1]
    return np.stack([_resize(cf, T) for cf in coeffs], axis=-1)


def _plan():
    """Shifted-banded matmul schedule with column compression.  Shifted
    k-block j covers rows [128j - SHIFT, 128j - SHIFT + 128); every t-chunk's
    band fits in blocks (j0, j0+1), and within each block only a contiguous
    column range [a, b) of the 512 chunk columns is nonzero.  Each matmul
    streams only its nonzero range; PSUM has_written semantics merge the two
    (start=True clears the bank, start=False accumulates where written and
    overwrites where not).  Packed M stores only the nonzero ranges,
    concatenated: [128, sum(b - a)].

    Rows are compressed too: each pair's matmul contracts only over its
    nonzero partition range [r_lo, r_hi) (r_lo 32-aligned for the PE base
    partition); excluded rows were zero so results are identical.  For the
    DMA side, pairs are bucketed into 3 row-range groups (rounded to
    [32,128) / [0,128) / [0,96)) and the packed-M columns are REORDERED so
    each group is contiguous -> a few big group DMAs instead of 32 small
    dispatch-bound ones.

    Returns (j0 per t-chunk,
             per-subdma (g_lo, g_hi, sbuf_col_off, ncols, trigger_tc, blob),
             per-pair (col_a, col_b, sbuf_col_off, r_lo, r_hi))."""
    M = _build_operator()                      # [k, t, c]
    Mi = M.reshape(T, T * C)                   # col = 4 t + c
    j0s = []
    raw = []               # per-pair (a, b, r_lo, r_hi, block[128, b-a])
    for tc in range(TCHUNKS):
        cols = Mi[:, tc * 512:(tc + 1) * 512]
        rows = np.nonzero(np.any(cols != 0, axis=1))[0]
        j0 = (rows.min() + SHIFT) // 128
        assert rows.max() < 128 * (j0 + 2) - SHIFT, (tc, rows.min(), rows.max())
        j0s.append(j0)
        bounds = []
        for i, j in enumerate((j0, j0 + 1)):
            k_lo = max(0, 128 * j - SHIFT)
            k_hi = min(T, 128 * (j + 1) - SHIFT)
            p_lo = k_lo - (128 * j - SHIFT)
            blk = np.zeros((128, 512))
            blk[p_lo:p_lo + (k_hi - k_lo)] = Mi[k_lo:k_hi, tc * 512:(tc + 1) * 512]
            nz = np.nonzero(np.any(blk != 0, axis=0))[0]
            a = (nz.min() // 8) * 8            # 8-col alignment (PSUM lines)
            b = min(512, (nz.max() // 8 + 1) * 8)
            nzr = np.nonzero(np.any(blk != 0, axis=1))[0]
            r_lo = (int(nzr.min()) // 32) * 32
            r_hi = int(nzr.max()) + 1
            raw.append((a, b, r_lo, r_hi, blk[:, a:b]))
            bounds.append((a, b))
        # the two ranges must tile [0, 512) with overlap only
        (a0, b0), (a1, b1) = bounds
        assert a0 == 0 and b1 == 512 and a1 <= b0, (tc, bounds)

    def group_of(p):
        a, b, r_lo, r_hi, blk = raw[p]
        return (r_lo, 96 if r_hi <= 96 else 128)

    order = sorted(range(2 * TCHUNKS), key=lambda p: (group_of(p), p))
    cranges = [None] * (2 * TCHUNKS)
    subdmas = []
    off = 0
    cur = None             # (g_lo, g_hi, start_off, cols, trigger, blobs)
    for p in order:
        a, b, r_lo, r_hi, blk = raw[p]
        g_lo, g_hi = group_of(p)
        cranges[p] = (a, b, off, r_lo, r_hi)
        if cur is None or (g_lo, g_hi) != tuple(cur[:2]) or cur[3] >= 1200:
            if cur is not None:
                subdmas.append(cur)
            cur = [g_lo, g_hi, off, 0, p // 2, []]
        cur[3] += b - a
        cur[5].append(blk[g_lo:g_hi])
        off += b - a
    subdmas.append(cur)
    subdmas = [(g_lo, g_hi, soff, ncols, trig,
                np.concatenate(blobs, axis=1).astype(np.float32))
               for (g_lo, g_hi, soff, ncols, trig, blobs) in subdmas]
    return j0s, subdmas, cranges


_J0S, _M_SUBDMAS, _CRANGES = _plan()
NPAIRS = 2 * TCHUNKS
MCOLS = sum(b - a for (a, b, _, _, _) in _CRANGES)

F32 = mybir.dt.float32
BF16 = mybir.dt.bfloat16

import ml_dtypes  # noqa: E402

_M_SUBDMAS = [(g_lo, g_hi, soff, ncols, trig, blob.astype(ml_dtypes.bfloat16))
              for (g_lo, g_hi, soff, ncols, trig, blob) in _M_SUBDMAS]


# ------------------------------------------------------------- device program
def _emit_body(tc_ctx, nc, x_d, m_d, o_d, pools, ogroup=None):
    mpool, spool, opool, ppool = pools

    m_t = mpool.tile([128, MCOLS], BF16, name="m_t")
    # M DMA: row+col-compressed group transfers, issued JIT at the first
    # t-chunk that needs them
    m_loaded = [False] * len(_M_SUBDMAS)

    def load_m(eng, tci):
        for s, (g_lo, g_hi, soff, ncols, trig, _blob) in enumerate(_M_SUBDMAS):
            if not m_loaded[s] and tci >= trig:
                eng.dma_start(m_t[g_lo:g_hi, soff:soff + ncols], m_d[s][:])
                m_loaded[s] = True

    if MPREFETCH == "top":
        # all M loads up-front on the ACT ring (idle until first outputs)
        for tci in range(TCHUNKS):
            load_m(nc.scalar, tci)
    ncopy = 0
    nout = 0
    sigw = 2 if SIGPAIR else 1  # signal blocks per sig tile/DMA
    for b in range(B_PER_CORE):
        for nbp in range(N // (128 * sigw)):
            sig = spool.tile([128, sigw, NBLK, 128], BF16, name="sig")
            nc.sync.dma_start(sig[:], x_d[b, nbp])
            for blk in range(sigw):
                nb = nbp * sigw + blk
                for tci in range(TCHUNKS):
                    # just-in-time M loads, emitted right before first use so
                    # the scheduler interleaves them with the first block's
                    # compute; resident afterwards
                    load_m(nc.sync, tci)
                    if PROBE == "dma":
                        # stores stream straight from m_t: loads + stores only
                        j = tci % ogroup
                        if j == ogroup - 1:
                            t0 = (tci - j) * 128
                            oeng = nc.scalar if nout % 2 == 0 else nc.sync
                            oeng.dma_start(
                                o_d[b, nb * 128:(nb + 1) * 128,
                                    t0:t0 + ogroup * 128, :],
                                m_t[:, 0:ogroup * 512])
                            nout += 1
                        continue
                    acc = ppool.tile([128, 512], F32, name="acc")
                    j0 = _J0S[tci]
                    a0, b0, off0, rl0, rh0 = _CRANGES[2 * tci]
                    a1, b1, off1, rl1, rh1 = _CRANGES[2 * tci + 1]
                    # col-compressed: each block streams only its nonzero
                    # range; start=True clears the bank, the second matmul
                    # accumulates on [a1, b0) and overwrites [b0, 512).
                    # Rows outside [r_lo, r_hi) are zero in M, so the
                    # contraction is restricted to that partition range.
                    nc.tensor.matmul(acc[:, a0:b0],
                                     sig[rl0:rh0, blk, j0, :],
                                     m_t[rl0:rh0, off0:off0 + (b0 - a0)],
                                     start=True, stop=False)
                    nc.tensor.matmul(acc[:, a1:b1],
                                     sig[rl1:rh1, blk, j0 + 1, :],
                                     m_t[rl1:rh1, off1:off1 + (b1 - a1)],
                                     start=False, stop=True)
                    if PROBE == "pe":
                        continue
                    j = tci % ogroup
                    if j == 0:
                        o_t = opool.tile([128, ogroup, 512], BF16, name="o_t")
                    if ncopy % COPY_SPLIT != COPY_SPLIT - 1:
                        nc.vector.tensor_copy(o_t[:, j, :], acc[:])
                    else:
                        nc.scalar.copy(o_t[:, j, :], acc[:])
                    ncopy += 1
                    if j == ogroup - 1:
                        # batched store (OGROUP t-chunks -> one DMA), off the
                        # SP ring (or alternating rings) so stores don't
                        # head-of-line block the input loads
                        t0 = (tci - j) * 128
                        oeng = nc.scalar
                        if OUT_RING == "alt" and nout % 2 == 1:
                            oeng = nc.sync
                        elif OUT_RING == "rot3":
                            oeng = (nc.scalar, nc.sync, nc.gpsimd)[nout % 3]
                        oeng.dma_start(
                            o_d[b, nb * 128:(nb + 1) * 128,
                                t0:t0 + ogroup * 128, :],
                            o_t[:],
                        )
                        nout += 1


def build_module(reps=1, ogroup=None):
    """Build + compile the per-core Bass module.  reps>1 wraps the body in a
    hardware loop (used by test.py for wall-clock differencing timing)."""
    if ogroup is None:
        ogroup = OGROUP
    nc = bacc.Bacc("TRN2", target_bir_lowering=False, debug=False)
    sigw = 2 if SIGPAIR else 1
    x_d = nc.dram_tensor("x", [B_PER_CORE, N // (128 * sigw), 128,
                               sigw * NBLK * 128],
                         BF16, kind="ExternalInput")
    m_d = [nc.dram_tensor(f"m{s}", list(sd[5].shape), BF16,
                          kind="ExternalInput")
           for s, sd in enumerate(_M_SUBDMAS)]
    o_d = nc.dram_tensor("out", [B_PER_CORE, N, T, C], BF16,
                         kind="ExternalOutput")

    with tile.TileContext(nc) as tc_ctx, ExitStack() as ctx:
        pools = (
            ctx.enter_context(tc_ctx.tile_pool(name="mpool", bufs=MBUFS)),
            ctx.enter_context(tc_ctx.tile_pool(name="spool", bufs=SBUFS)),
            ctx.enter_context(tc_ctx.tile_pool(name="opool",
                                             bufs=OBUFS or max(3, 24 // ogroup))),
            ctx.enter_context(tc_ctx.tile_pool(name="ppool", bufs=8, space="PSUM")),
        )
        if reps == 1:
            _emit_body(tc_ctx, nc, x_d, m_d, o_d, pools, ogroup)
        else:
            with tc_ctx.For_i(0, reps, 1,
                              hint_engines=(mybir.EngineType.PE,
                                            mybir.EngineType.SP),
                              staggered_reset=bool(STAGGER)):
                _emit_body(tc_ctx, nc, x_d, m_d, o_d, pools, ogroup)

    nc.compile()
    return nc


_NC_CACHE = {}


def _get_module(reps=1, ogroup=None):
    key = (reps, ogroup)
    if key not in _NC_CACHE:
        _NC_CACHE[key] = build_module(reps, ogroup)
    return _NC_CACHE[key]


# ------------------------------------------------------------------ entrypoint
def run(x, reps=1, ogroup=None):
    """x: [16, 2048, 512, 1] float32 -> [16, 512, 2048, 4] float32."""
    nc = _get_module(reps, ogroup)
    x3 = np.asarray(x)[:, :, :, 0].astype(ml_dtypes.bfloat16)  # [B, T, N]
    # shifted k grid: block j, partition p holds k = 128j - SHIFT + p
    xp = np.zeros((B, NBLK * 128, N), dtype=ml_dtypes.bfloat16)
    xp[:, SHIFT:SHIFT + T, :] = x3
    # pre-tile to the SBUF layout: [b, nb_group, p, (blk j n)]
    sigw = 2 if SIGPAIR else 1
    xt = np.ascontiguousarray(
        xp.reshape(B, NBLK, 128, N // (128 * sigw), sigw, 128)
        .transpose(0, 3, 2, 4, 1, 5)
        .reshape(B, N // (128 * sigw), 128, sigw * NBLK * 128))
    in_maps = [
        {"x": xt[c * B_PER_CORE:(c + 1) * B_PER_CORE],
         **{f"m{s}": sd[5] for s, sd in enumerate(_M_SUBDMAS)}}
        for c in range(NCORES)
    ]
    res = run_bass_kernel_spmd(nc, in_maps, core_ids=list(range(NCORES)))
    out = np.concatenate([res.results[c]["out"] for c in range(NCORES)], axis=0)
    return out.astype(np.float32)


def kernel(x):
    return run(x)



# revision 21
# speedup vs baseline: 1.6885x; 1.6885x over previous
"""Trainium2 Bass kernel for nn_DiscreteWaveletTransform (3-level db4 DWT,
symmetric padding, + linear resize of each coefficient band back to T).

Approach: the whole per-signal pipeline (3 DWT levels + 4 resizes) is one
fixed linear operator out[t, c] = sum_k sig[k] * M[k, 4t+c].  M (2048 x 8192)
is banded: the 512 output columns of one 128-wide t-chunk depend on <= 194
consecutive k rows, and the band start advances by exactly 128 k per t-chunk.
Blocking k on a grid shifted by SHIFT (so every t-chunk's band falls in two
consecutive 128-row blocks) gives exactly 2 PE matmuls per t-chunk:
[K=128, M=128 signals, N<=512 cols] in bf16, accumulated in PSUM.  Within
each 128-row block only a contiguous column range is nonzero, so each matmul
streams just that range (~71% of 2x512 total) and packed M stores only those
ranges; PSUM has_written semantics stitch the two ranges together.  The matmul
orientation puts signals on PSUM partitions and (t, c)-interleaved columns on
the free axis, so each PSUM tile is a contiguous [128 signals, 128 t x 4 c]
chunk of the output layout — no transposes, contiguous output DMAs.

The output is stored to DRAM in bf16 (halves the dominant HBM stream) and
upcast to float32 on the host during the gather.

Sharding: data-parallel over B (16 -> 2 per core); each core handles
2 x 512 = 1024 signals = 8 blocks of 128.
"""

from contextlib import ExitStack

import numpy as np

import concourse.bacc as bacc
import concourse.bass as bass
import concourse.tile as tile
from concourse import mybir
from concourse.bass_utils import run_bass_kernel_spmd

# ---------------------------------------------------------------- problem dims
import os  # noqa: E402

B, T, N = 16, 2048, 512
LEVELS = 3
C = LEVELS + 1
F = 8
NCORES = 8
B_PER_CORE = B // NCORES          # 2
TCHUNKS = T // 128                # 16
SBLOCKS = B_PER_CORE * (N // 128)  # 8 signal blocks of 128 per core
OGROUP = int(os.environ.get("K_OGROUP", "8"))  # t-chunks per output DMA
OUT_RING = os.environ.get("K_OUT_RING", "alt")   # act | alt  (output DMA queue)
MPREFETCH = os.environ.get("K_MPREFETCH", "jit")  # jit | top  (M load placement)
COPY_SPLIT = int(os.environ.get("K_COPY_SPLIT", "2"))  # n-1 of n copies on DVE
OBUFS = int(os.environ.get("K_OBUFS", "0"))      # opool bufs override (0=auto)
MCOMPRESS = int(os.environ.get("K_MCOMPRESS", "0"))  # DMA/matmul only nonzero M rows
SIGSPLIT = int(os.environ.get("K_SIGSPLIT", "0"))    # split first sig load
PROBE = os.environ.get("K_PROBE", "full")  # full | dma | pe  (timing probes)
STAGGER = int(os.environ.get("K_STAGGER", "1"))  # staggered sem reset in For_i
MGROUP = int(os.environ.get("K_MGROUP", "2"))    # M pairs per load DMA (2|8|32)
SIGPAIR = int(os.environ.get("K_SIGPAIR", "0"))  # 2 signal blocks per sig DMA
SBUFS = int(os.environ.get("K_SBUFS", "3"))      # spool bufs (sig prefetch depth)
MBUFS = int(os.environ.get("K_MBUFS", "1"))      # mpool bufs (cross-rep M decouple)
SHIFT = 60                         # k-grid shift aligning blocks to the band
NBLK = (T + SHIFT + 127) // 128    # 17 shifted k-blocks cover [-SHIFT, 2116)

DEC_LO = np.array([-0.010597401784997278, 0.032883011666982945, 0.030841381835986965,
                   -0.18703481171888114, -0.02798376941698385, 0.6308807679295904,
                   0.7148465705525415, 0.23037781330885523])
DEC_HI = np.array([-0.23037781330885523, 0.7148465705525415, -0.6308807679295904,
                   -0.02798376941698385, 0.18703481171888114, 0.030841381835986965,
                   -0.032883011666982945, -0.010597401784997278])


# ------------------------------------------------------- operator construction
def _dwt_step(sig, lo, hi):
    S = sig.shape[1]
    ext = np.pad(sig, ((0, 0), (F - 1, F - 1)), mode='symmetric')[:, 1:]
    L = (S + F - 1) // 2
    lo_r, hi_r = lo[::-1], hi[::-1]
    cA = sum(ext[:, k:k + 2 * L:2] * lo_r[k] for k in range(F))
    cD = sum(ext[:, k:k + 2 * L:2] * hi_r[k] for k in range(F))
    return cA, cD


def _resize(c, t):
    S = c.shape[-1]
    if S == t:
        return c
    if S > t:
        return c[..., :t]
    pos = (np.arange(t, dtype=c.dtype) + 0.5) * (S / t) - 0.5
    pos = np.clip(pos, 0.0, S - 1)
    lo = np.floor(pos).astype(np.int64)
    hi = np.minimum(lo + 1, S - 1)
    w = pos - lo.astype(c.dtype)
    return c[..., lo] * (1.0 - w) + c[..., hi] * w


def _build_operator():
    """M [T, T, C] float64: out[s, t, c] = sum_k sig[s, k] M[k, t, c]."""
    a = np.eye(T)
    details = []
    for _ in range(LEVELS):
        a, d = _dwt_step(a, DEC_LO, DEC_HI)
        details.append(d)
    coeffs = [a] + details[::-1]
    return np.stack([_resize(cf, T) for cf in coeffs], axis=-1)


def _plan():
    """Shifted-banded matmul schedule with column compression.  Shifted
    k-block j covers rows [128j - SHIFT, 128j - SHIFT + 128); every t-chunk's
    band fits in blocks (j0, j0+1), and within each block only a contiguous
    column range [a, b) of the 512 chunk columns is nonzero.  Each matmul
    streams only its nonzero range; PSUM has_written semantics merge the two
    (start=True clears the bank, start=False accumulates where written and
    overwrites where not).  Packed M stores only the nonzero ranges,
    concatenated: [128, sum(b - a)].

    Rows are compressed too: each pair's matmul contracts only over its
    nonzero partition range [r_lo, r_hi) (r_lo 32-aligned for the PE base
    partition); excluded rows were zero so results are identical.  For the
    DMA side, pairs are bucketed into 3 row-range groups (rounded to
    [32,128) / [0,128) / [0,96)) and the packed-M columns are REORDERED so
    each group is contiguous -> a few big group DMAs instead of 32 small
    dispatch-bound ones.

    Returns (j0 per t-chunk,
             per-subdma (g_lo, g_hi, sbuf_col_off, ncols, trigger_tc, blob),
             per-pair (col_a, col_b, sbuf_col_off, r_lo, r_hi))."""
    M = _build_operator()                      # [k, t, c]
    Mi = M.reshape(T, T * C)                   # col = 4 t + c
    j0s = []
    raw = []               # per-pair (a, b, r_lo, r_hi, block[128, b-a])
    for tc in range(TCHUNKS):
        cols = Mi[:, tc * 512:(tc + 1) * 512]
        rows = np.nonzero(np.any(cols != 0, axis=1))[0]
        j0 = (rows.min() + SHIFT) // 128
        assert rows.max() < 128 * (j0 + 2) - SHIFT, (tc, rows.min(), rows.max())
        j0s.append(j0)
        bounds = []
        for i, j in enumerate((j0, j0 + 1)):
            k_lo = max(0, 128 * j - SHIFT)
            k_hi = min(T, 128 * (j + 1) - SHIFT)
            p_lo = k_lo - (128 * j - SHIFT)
            blk = np.zeros((128, 512))
            blk[p_lo:p_lo + (k_hi - k_lo)] = Mi[k_lo:k_hi, tc * 512:(tc + 1) * 512]
            nz = np.nonzero(np.any(blk != 0, axis=0))[0]
            a = (nz.min() // 8) * 8            # 8-col alignment (PSUM lines)
            b = min(512, (nz.max() // 8 + 1) * 8)
            nzr = np.nonzero(np.any(blk != 0, axis=1))[0]
            r_lo = (int(nzr.min()) // 32) * 32
            r_hi = int(nzr.max()) + 1
            raw.append((a, b, r_lo, r_hi, blk[:, a:b]))
            bounds.append((a, b))
        # the two ranges must tile [0, 512) with overlap only
        (a0, b0), (a1, b1) = bounds
        assert a0 == 0 and b1 == 512 and a1 <= b0, (tc, bounds)

    def group_of(p):
        a, b, r_lo, r_hi, blk = raw[p]
        return (r_lo, 96 if r_hi <= 96 else 128)

    order = sorted(range(2 * TCHUNKS), key=lambda p: (group_of(p), p))
    cranges = [None] * (2 * TCHUNKS)
    subdmas = []
    off = 0
    cur = None             # (g_lo, g_hi, start_off, cols, trigger, blobs)
    for p in order:
        a, b, r_lo, r_hi, blk = raw[p]
        g_lo, g_hi = group_of(p)
        cranges[p] = (a, b, off, r_lo, r_hi)
        if cur is None or (g_lo, g_hi) != tuple(cur[:2]) or cur[3] >= 1200:
            if cur is not None:
                subdmas.append(cur)
            cur = [g_lo, g_hi, off, 0, p // 2, []]
        cur[3] += b - a
        cur[5].append(blk[g_lo:g_hi])
        off += b - a
    subdmas.append(cur)
    subdmas = [(g_lo, g_hi, soff, ncols, trig,
                np.concatenate(blobs, axis=1).astype(np.float32))
               for (g_lo, g_hi, soff, ncols, trig, blobs) in subdmas]
    return j0s, subdmas, cranges


_J0S, _M_SUBDMAS, _CRANGES = _plan()
NPAIRS = 2 * TCHUNKS
MCOLS = sum(b - a for (a, b, _, _, _) in _CRANGES)

F32 = mybir.dt.float32
BF16 = mybir.dt.bfloat16

import ml_dtypes  # noqa: E402

_M_SUBDMAS = [(g_lo, g_hi, soff, ncols, trig, blob.astype(ml_dtypes.bfloat16))
              for (g_lo, g_hi, soff, ncols, trig, blob) in _M_SUBDMAS]


# ------------------------------------------------------------- device program
def _emit_body(tc_ctx, nc, x_d, m_d, o_d, pools, ogroup=None):
    mpool, spool, opool, ppool = pools

    m_t = mpool.tile([128, MCOLS], BF16, name="m_t")
    # M DMA: row+col-compressed group transfers, issued JIT at the first
    # t-chunk that needs them
    m_loaded = [False] * len(_M_SUBDMAS)

    # LDWEIGHTS must start at partition 0, so rows [0, g_lo) of the
    # g_lo > 0 groups' columns are zeroed once upfront (idle Pool engine,
    # overlaps the first sig DMA) and every contraction starts at partition 0
    zr = [(g_lo, soff, ncols) for (g_lo, _, soff, ncols, _, _) in _M_SUBDMAS
          if g_lo > 0]
    if zr:
        z_gl = max(g for g, _, _ in zr)
        z_lo = min(s for _, s, _ in zr)
        z_hi = max(s + n for _, s, n in zr)
        nc.gpsimd.memset(m_t[0:z_gl, z_lo:z_hi], 0)

    def load_m(eng, tci):
        for s, (g_lo, g_hi, soff, ncols, trig, _blob) in enumerate(_M_SUBDMAS):
            if not m_loaded[s] and tci >= trig:
                eng.dma_start(m_t[g_lo:g_hi, soff:soff + ncols], m_d[s][:])
                m_loaded[s] = True

    if MPREFETCH == "top":
        # all M loads up-front on the ACT ring (idle until first outputs)
        for tci in range(TCHUNKS):
            load_m(nc.scalar, tci)
    ncopy = 0
    nout = 0
    sigw = 2 if SIGPAIR else 1  # signal blocks per sig tile/DMA
    for b in range(B_PER_CORE):
        for nbp in range(N // (128 * sigw)):
            sig = spool.tile([128, sigw, NBLK, 128], BF16, name="sig")
            nc.sync.dma_start(sig[:], x_d[b, nbp])
            for blk in range(sigw):
                nb = nbp * sigw + blk
                for tci in range(TCHUNKS):
                    # just-in-time M loads, emitted right before first use so
                    # the scheduler interleaves them with the first block's
                    # compute; resident afterwards
                    load_m(nc.sync, tci)
                    if PROBE == "dma":
                        # stores stream straight from m_t: loads + stores only
                        j = tci % ogroup
                        if j == ogroup - 1:
                            t0 = (tci - j) * 128
                            oeng = nc.scalar if nout % 2 == 0 else nc.sync
                            oeng.dma_start(
                                o_d[b, nb * 128:(nb + 1) * 128,
                                    t0:t0 + ogroup * 128, :],
                                m_t[:, 0:ogroup * 512])
                            nout += 1
                        continue
                    acc = ppool.tile([128, 512], F32, name="acc")
                    j0 = _J0S[tci]
                    a0, b0, off0, rl0, rh0 = _CRANGES[2 * tci]
                    a1, b1, off1, rl1, rh1 = _CRANGES[2 * tci + 1]
                    # col-compressed: each block streams only its nonzero
                    # range; start=True clears the bank, the second matmul
                    # accumulates on [a1, b0) and overwrites [b0, 512).
                    # Rows outside [r_lo, r_hi) are zero in M, so the
                    # contraction is restricted to that partition range.
                    nc.tensor.matmul(acc[:, a0:b0],
                                     sig[0:rh0, blk, j0, :],
                                     m_t[0:rh0, off0:off0 + (b0 - a0)],
                                     start=True, stop=False)
                    nc.tensor.matmul(acc[:, a1:b1],
                                     sig[0:rh1, blk, j0 + 1, :],
                                     m_t[0:rh1, off1:off1 + (b1 - a1)],
                                     start=False, stop=True)
                    if PROBE == "pe":
                        continue
                    j = tci % ogroup
                    if j == 0:
                        o_t = opool.tile([128, ogroup, 512], BF16, name="o_t")
                    if ncopy % COPY_SPLIT != COPY_SPLIT - 1:
                        nc.vector.tensor_copy(o_t[:, j, :], acc[:])
                    else:
                        nc.scalar.copy(o_t[:, j, :], acc[:])
                    ncopy += 1
                    if j == ogroup - 1:
                        # batched store (OGROUP t-chunks -> one DMA), off the
                        # SP ring (or alternating rings) so stores don't
                        # head-of-line block the input loads
                        t0 = (tci - j) * 128
                        oeng = nc.scalar
                        if OUT_RING == "alt" and nout % 2 == 1:
                            oeng = nc.sync
                        elif OUT_RING == "rot3":
                            oeng = (nc.scalar, nc.sync, nc.gpsimd)[nout % 3]
                        oeng.dma_start(
                            o_d[b, nb * 128:(nb + 1) * 128,
                                t0:t0 + ogroup * 128, :],
                            o_t[:],
                        )
                        nout += 1


def build_module(reps=1, ogroup=None):
    """Build + compile the per-core Bass module.  reps>1 wraps the body in a
    hardware loop (used by test.py for wall-clock differencing timing)."""
    if ogroup is None:
        ogroup = OGROUP
    nc = bacc.Bacc("TRN2", target_bir_lowering=False, debug=False)
    sigw = 2 if SIGPAIR else 1
    x_d = nc.dram_tensor("x", [B_PER_CORE, N // (128 * sigw), 128,
                               sigw * NBLK * 128],
                         BF16, kind="ExternalInput")
    m_d = [nc.dram_tensor(f"m{s}", list(sd[5].shape), BF16,
                          kind="ExternalInput")
           for s, sd in enumerate(_M_SUBDMAS)]
    o_d = nc.dram_tensor("out", [B_PER_CORE, N, T, C], BF16,
                         kind="ExternalOutput")

    with tile.TileContext(nc) as tc_ctx, ExitStack() as ctx:
        pools = (
            ctx.enter_context(tc_ctx.tile_pool(name="mpool", bufs=MBUFS)),
            ctx.enter_context(tc_ctx.tile_pool(name="spool", bufs=SBUFS)),
            ctx.enter_context(tc_ctx.tile_pool(name="opool",
                                             bufs=OBUFS or max(3, 24 // ogroup))),
            ctx.enter_context(tc_ctx.tile_pool(name="ppool", bufs=8, space="PSUM")),
        )
        if reps == 1:
            _emit_body(tc_ctx, nc, x_d, m_d, o_d, pools, ogroup)
        else:
            with tc_ctx.For_i(0, reps, 1,
                              hint_engines=(mybir.EngineType.PE,
                                            mybir.EngineType.SP),
                              staggered_reset=bool(STAGGER)):
                _emit_body(tc_ctx, nc, x_d, m_d, o_d, pools, ogroup)

    nc.compile()
    return nc


_NC_CACHE = {}


def _get_module(reps=1, ogroup=None):
    key = (reps, ogroup)
    if key not in _NC_CACHE:
        _NC_CACHE[key] = build_module(reps, ogroup)
    return _NC_CACHE[key]


# ------------------------------------------------------------------ entrypoint
def run(x, reps=1, ogroup=None):
    """x: [16, 2048, 512, 1] float32 -> [16, 512, 2048, 4] float32."""
    nc = _get_module(reps, ogroup)
    x3 = np.asarray(x)[:, :, :, 0].astype(ml_dtypes.bfloat16)  # [B, T, N]
    # shifted k grid: block j, partition p holds k = 128j - SHIFT + p
    xp = np.zeros((B, NBLK * 128, N), dtype=ml_dtypes.bfloat16)
    xp[:, SHIFT:SHIFT + T, :] = x3
    # pre-tile to the SBUF layout: [b, nb_group, p, (blk j n)]
    sigw = 2 if SIGPAIR else 1
    xt = np.ascontiguousarray(
        xp.reshape(B, NBLK, 128, N // (128 * sigw), sigw, 128)
        .transpose(0, 3, 2, 4, 1, 5)
        .reshape(B, N // (128 * sigw), 128, sigw * NBLK * 128))
    in_maps = [
        {"x": xt[c * B_PER_CORE:(c + 1) * B_PER_CORE],
         **{f"m{s}": sd[5] for s, sd in enumerate(_M_SUBDMAS)}}
        for c in range(NCORES)
    ]
    res = run_bass_kernel_spmd(nc, in_maps, core_ids=list(range(NCORES)))
    out = np.concatenate([res.results[c]["out"] for c in range(NCORES)], axis=0)
    return out.astype(np.float32)


def kernel(x):
    return run(x)



# revision 24
# speedup vs baseline: 1.7505x; 1.0367x over previous
"""Trainium2 Bass kernel for nn_DiscreteWaveletTransform (3-level db4 DWT,
symmetric padding, + linear resize of each coefficient band back to T).

Approach: the whole per-signal pipeline (3 DWT levels + 4 resizes) is one
fixed linear operator out[t, c] = sum_k sig[k] * M[k, 4t+c].  M (2048 x 8192)
is banded: the 512 output columns of one 128-wide t-chunk depend on <= 194
consecutive k rows, and the band start advances by exactly 128 k per t-chunk.
Blocking k on a grid shifted by SHIFT (so every t-chunk's band falls in two
consecutive 128-row blocks) gives exactly 2 PE matmuls per t-chunk:
[K=128, M=128 signals, N<=512 cols] in bf16, accumulated in PSUM.  Within
each 128-row block only a contiguous column range is nonzero, so each matmul
streams just that range (~71% of 2x512 total) and packed M stores only those
ranges; PSUM has_written semantics stitch the two ranges together.  The matmul
orientation puts signals on PSUM partitions and (t, c)-interleaved columns on
the free axis, so each PSUM tile is a contiguous [128 signals, 128 t x 4 c]
chunk of the output layout — no transposes, contiguous output DMAs.

The output is stored to DRAM in bf16 (halves the dominant HBM stream) and
upcast to float32 on the host during the gather.

Sharding: data-parallel over B (16 -> 2 per core); each core handles
2 x 512 = 1024 signals = 8 blocks of 128.
"""

from contextlib import ExitStack

import numpy as np

import concourse.bacc as bacc
import concourse.bass as bass
import concourse.tile as tile
from concourse import mybir
from concourse.bass_utils import run_bass_kernel_spmd

# ---------------------------------------------------------------- problem dims
import os  # noqa: E402

B, T, N = 16, 2048, 512
LEVELS = 3
C = LEVELS + 1
F = 8
NCORES = 8
B_PER_CORE = B // NCORES          # 2
TCHUNKS = T // 128                # 16
SBLOCKS = B_PER_CORE * (N // 128)  # 8 signal blocks of 128 per core
OGROUP = int(os.environ.get("K_OGROUP", "8"))  # t-chunks per output DMA
OUT_RING = os.environ.get("K_OUT_RING", "alt")   # act | alt  (output DMA queue)
MPREFETCH = os.environ.get("K_MPREFETCH", "jit")  # jit | top  (M load placement)
COPY_SPLIT = int(os.environ.get("K_COPY_SPLIT", "2"))  # n-1 of n copies on DVE
OBUFS = int(os.environ.get("K_OBUFS", "0"))      # opool bufs override (0=auto)
MCOMPRESS = int(os.environ.get("K_MCOMPRESS", "0"))  # DMA/matmul only nonzero M rows
SIGSPLIT = int(os.environ.get("K_SIGSPLIT", "0"))    # split first sig load
PROBE = os.environ.get("K_PROBE", "full")  # full | dma | pe  (timing probes)
STAGGER = int(os.environ.get("K_STAGGER", "1"))  # staggered sem reset in For_i
MGROUP = int(os.environ.get("K_MGROUP", "2"))    # M pairs per load DMA (2|8|32)
SIGPAIR = int(os.environ.get("K_SIGPAIR", "0"))  # 2 signal blocks per sig DMA
SBUFS = int(os.environ.get("K_SBUFS", "3"))      # spool bufs (sig prefetch depth)
MBUFS = int(os.environ.get("K_MBUFS", "1"))      # mpool bufs (cross-rep M decouple)
PBANKS = int(os.environ.get("K_PBANKS", "2"))    # t-chunks (PSUM banks) per copy
SHIFT = 60                         # k-grid shift aligning blocks to the band
NBLK = (T + SHIFT + 127) // 128    # 17 shifted k-blocks cover [-SHIFT, 2116)

DEC_LO = np.array([-0.010597401784997278, 0.032883011666982945, 0.030841381835986965,
                   -0.18703481171888114, -0.02798376941698385, 0.6308807679295904,
                   0.7148465705525415, 0.23037781330885523])
DEC_HI = np.array([-0.23037781330885523, 0.7148465705525415, -0.6308807679295904,
                   -0.02798376941698385, 0.18703481171888114, 0.030841381835986965,
                   -0.032883011666982945, -0.010597401784997278])


# ------------------------------------------------------- operator construction
def _dwt_step(sig, lo, hi):
    S = sig.shape[1]
    ext = np.pad(sig, ((0, 0), (F - 1, F - 1)), mode='symmetric')[:, 1:]
    L = (S + F - 1) // 2
    lo_r, hi_r = lo[::-1], hi[::-1]
    cA = sum(ext[:, k:k + 2 * L:2] * lo_r[k] for k in range(F))
    cD = sum(ext[:, k:k + 2 * L:2] * hi_r[k] for k in range(F))
    return cA, cD


def _resize(c, t):
    S = c.shape[-1]
    if S == t:
        return c
    if S > t:
        return c[..., :t]
    pos = (np.arange(t, dtype=c.dtype) + 0.5) * (S / t) - 0.5
    pos = np.clip(pos, 0.0, S - 1)
    lo = np.floor(pos).astype(np.int64)
    hi = np.minimum(lo + 1, S - 1)
    w = pos - lo.astype(c.dtype)
    return c[..., lo] * (1.0 - w) + c[..., hi] * w


def _build_operator():
    """M [T, T, C] float64: out[s, t, c] = sum_k sig[s, k] M[k, t, c]."""
    a = np.eye(T)
    details = []
    for _ in range(LEVELS):
        a, d = _dwt_step(a, DEC_LO, DEC_HI)
        details.append(d)
    coeffs = [a] + details[::-1]
    return np.stack([_resize(cf, T) for cf in coeffs], axis=-1)


def _plan():
    """Shifted-banded matmul schedule with column compression.  Shifted
    k-block j covers rows [128j - SHIFT, 128j - SHIFT + 128); every t-chunk's
    band fits in blocks (j0, j0+1), and within each block only a contiguous
    column range [a, b) of the 512 chunk columns is nonzero.  Each matmul
    streams only its nonzero range; PSUM has_written semantics merge the two
    (start=True clears the bank, start=False accumulates where written and
    overwrites where not).  Packed M stores only the nonzero ranges,
    concatenated: [128, sum(b - a)].

    Rows are compressed too: each pair's matmul contracts only over its
    nonzero partition range [r_lo, r_hi) (r_lo 32-aligned for the PE base
    partition); excluded rows were zero so results are identical.  For the
    DMA side, pairs are bucketed into 3 row-range groups (rounded to
    [32,128) / [0,128) / [0,96)) and the packed-M columns are REORDERED so
    each group is contiguous -> a few big group DMAs instead of 32 small
    dispatch-bound ones.

    Returns (j0 per t-chunk,
             per-subdma (g_lo, g_hi, sbuf_col_off, ncols, trigger_tc, blob),
             per-pair (col_a, col_b, sbuf_col_off, r_lo, r_hi))."""
    M = _build_operator()                      # [k, t, c]
    Mi = M.reshape(T, T * C)                   # col = 4 t + c
    j0s = []
    raw = []               # per-pair (a, b, r_lo, r_hi, block[128, b-a])
    for tc in range(TCHUNKS):
        cols = Mi[:, tc * 512:(tc + 1) * 512]
        rows = np.nonzero(np.any(cols != 0, axis=1))[0]
        j0 = (rows.min() + SHIFT) // 128
        assert rows.max() < 128 * (j0 + 2) - SHIFT, (tc, rows.min(), rows.max())
        j0s.append(j0)
        bounds = []
        for i, j in enumerate((j0, j0 + 1)):
            k_lo = max(0, 128 * j - SHIFT)
            k_hi = min(T, 128 * (j + 1) - SHIFT)
            p_lo = k_lo - (128 * j - SHIFT)
            blk = np.zeros((128, 512))
            blk[p_lo:p_lo + (k_hi - k_lo)] = Mi[k_lo:k_hi, tc * 512:(tc + 1) * 512]
            nz = np.nonzero(np.any(blk != 0, axis=0))[0]
            a = (nz.min() // 8) * 8            # 8-col alignment (PSUM lines)
            b = min(512, (nz.max() // 8 + 1) * 8)
            nzr = np.nonzero(np.any(blk != 0, axis=1))[0]
            r_lo = (int(nzr.min()) // 32) * 32
            r_hi = int(nzr.max()) + 1
            raw.append((a, b, r_lo, r_hi, blk[:, a:b]))
            bounds.append((a, b))
        # the two ranges must tile [0, 512) with overlap only
        (a0, b0), (a1, b1) = bounds
        assert a0 == 0 and b1 == 512 and a1 <= b0, (tc, bounds)

    def group_of(p):
        a, b, r_lo, r_hi, blk = raw[p]
        return (r_lo, 96 if r_hi <= 96 else 128)

    order = sorted(range(2 * TCHUNKS), key=lambda p: (group_of(p), p))
    cranges = [None] * (2 * TCHUNKS)
    subdmas = []
    off = 0
    cur = None             # (g_lo, g_hi, start_off, cols, trigger, blobs)
    for p in order:
        a, b, r_lo, r_hi, blk = raw[p]
        g_lo, g_hi = group_of(p)
        cranges[p] = (a, b, off, r_lo, r_hi)
        if cur is None or (g_lo, g_hi) != tuple(cur[:2]) or cur[3] >= 1200:
            if cur is not None:
                subdmas.append(cur)
            cur = [g_lo, g_hi, off, 0, p // 2, []]
        cur[3] += b - a
        cur[5].append(blk[g_lo:g_hi])
        off += b - a
    subdmas.append(cur)
    subdmas = [(g_lo, g_hi, soff, ncols, trig,
                np.concatenate(blobs, axis=1).astype(np.float32))
               for (g_lo, g_hi, soff, ncols, trig, blobs) in subdmas]
    return j0s, subdmas, cranges


_J0S, _M_SUBDMAS, _CRANGES = _plan()
NPAIRS = 2 * TCHUNKS
MCOLS = sum(b - a for (a, b, _, _, _) in _CRANGES)

F32 = mybir.dt.float32
BF16 = mybir.dt.bfloat16

import ml_dtypes  # noqa: E402

_M_SUBDMAS = [(g_lo, g_hi, soff, ncols, trig, blob.astype(ml_dtypes.bfloat16))
              for (g_lo, g_hi, soff, ncols, trig, blob) in _M_SUBDMAS]


# ------------------------------------------------------------- device program
def _emit_body(tc_ctx, nc, x_d, m_d, o_d, pools, ogroup=None):
    mpool, spool, opool, ppool = pools

    m_t = mpool.tile([128, MCOLS], BF16, name="m_t")
    # M DMA: row+col-compressed group transfers, issued JIT at the first
    # t-chunk that needs them
    m_loaded = [False] * len(_M_SUBDMAS)

    # LDWEIGHTS must start at partition 0, so rows [0, g_lo) of the
    # g_lo > 0 groups' columns are zeroed once upfront (idle Pool engine,
    # overlaps the first sig DMA) and every contraction starts at partition 0
    zr = [(g_lo, soff, ncols) for (g_lo, _, soff, ncols, _, _) in _M_SUBDMAS
          if g_lo > 0]
    if zr:
        z_gl = max(g for g, _, _ in zr)
        z_lo = min(s for _, s, _ in zr)
        z_hi = max(s + n for _, s, n in zr)
        nc.gpsimd.memset(m_t[0:z_gl, z_lo:z_hi], 0)

    def load_m(eng, tci):
        for s, (g_lo, g_hi, soff, ncols, trig, _blob) in enumerate(_M_SUBDMAS):
            if not m_loaded[s] and tci >= trig:
                eng.dma_start(m_t[g_lo:g_hi, soff:soff + ncols], m_d[s][:])
                m_loaded[s] = True

    if MPREFETCH == "top":
        # all M loads up-front on the ACT ring (idle until first outputs)
        for tci in range(TCHUNKS):
            load_m(nc.scalar, tci)
    ncopy = 0
    nout = 0
    sigw = 2 if SIGPAIR else 1  # signal blocks per sig tile/DMA
    for b in range(B_PER_CORE):
        for nbp in range(N // (128 * sigw)):
            sig = spool.tile([128, sigw, NBLK, 128], BF16, name="sig")
            nc.sync.dma_start(sig[:], x_d[b, nbp])
            for blk in range(sigw):
                nb = nbp * sigw + blk
                for tci in range(TCHUNKS):
                    # just-in-time M loads, emitted right before first use so
                    # the scheduler interleaves them with the first block's
                    # compute; resident afterwards
                    load_m(nc.sync, tci)
                    if PROBE == "dma":
                        # stores stream straight from m_t: loads + stores only
                        j = tci % ogroup
                        if j == ogroup - 1:
                            t0 = (tci - j) * 128
                            oeng = nc.scalar if nout % 2 == 0 else nc.sync
                            oeng.dma_start(
                                o_d[b, nb * 128:(nb + 1) * 128,
                                    t0:t0 + ogroup * 128, :],
                                m_t[:, 0:ogroup * 512])
                            nout += 1
                        continue
                    if tci % PBANKS == 0:
                        acc = ppool.tile([128, PBANKS, 512], F32, name="acc")
                    q = tci % PBANKS
                    j0 = _J0S[tci]
                    a0, b0, off0, rl0, rh0 = _CRANGES[2 * tci]
                    a1, b1, off1, rl1, rh1 = _CRANGES[2 * tci + 1]
                    # col-compressed: each block streams only its nonzero
                    # range; start=True clears the bank, the second matmul
                    # accumulates on [a1, b0) and overwrites [b0, 512).
                    # Rows outside [r_lo, r_hi) are zero in M, so the
                    # contraction is restricted to that partition range.
                    nc.tensor.matmul(acc[:, q, a0:b0],
                                     sig[0:rh0, blk, j0, :],
                                     m_t[0:rh0, off0:off0 + (b0 - a0)],
                                     start=True, stop=False)
                    nc.tensor.matmul(acc[:, q, a1:b1],
                                     sig[0:rh1, blk, j0 + 1, :],
                                     m_t[0:rh1, off1:off1 + (b1 - a1)],
                                     start=False, stop=True)
                    if PROBE == "pe":
                        continue
                    j = tci % ogroup
                    if j == 0:
                        o_t = opool.tile([128, ogroup, 512], BF16, name="o_t")
                    if q == PBANKS - 1:
                        # one copy per PBANKS accumulated banks (contiguous
                        # PSUM read) -> fewer, longer copy instructions
                        dst = o_t[:, j - PBANKS + 1:j + 1, :]
                        if ncopy % COPY_SPLIT != COPY_SPLIT - 1:
                            nc.vector.tensor_copy(dst, acc[:])
                        else:
                            nc.scalar.copy(dst, acc[:])
                        ncopy += 1
                    if j == ogroup - 1:
                        # batched store (OGROUP t-chunks -> one DMA), off the
                        # SP ring (or alternating rings) so stores don't
                        # head-of-line block the input loads
                        t0 = (tci - j) * 128
                        oeng = nc.scalar
                        if OUT_RING == "alt" and nout % 2 == 1:
                            oeng = nc.sync
                        elif OUT_RING == "rot3":
                            oeng = (nc.scalar, nc.sync, nc.gpsimd)[nout % 3]
                        oeng.dma_start(
                            o_d[b, nb * 128:(nb + 1) * 128,
                                t0:t0 + ogroup * 128, :],
                            o_t[:],
                        )
                        nout += 1


def build_module(reps=1, ogroup=None):
    """Build + compile the per-core Bass module.  reps>1 wraps the body in a
    hardware loop (used by test.py for wall-clock differencing timing)."""
    if ogroup is None:
        ogroup = OGROUP
    nc = bacc.Bacc("TRN2", target_bir_lowering=False, debug=False)
    sigw = 2 if SIGPAIR else 1
    x_d = nc.dram_tensor("x", [B_PER_CORE, N // (128 * sigw), 128,
                               sigw * NBLK * 128],
                         BF16, kind="ExternalInput")
    m_d = [nc.dram_tensor(f"m{s}", list(sd[5].shape), BF16,
                          kind="ExternalInput")
           for s, sd in enumerate(_M_SUBDMAS)]
    o_d = nc.dram_tensor("out", [B_PER_CORE, N, T, C], BF16,
                         kind="ExternalOutput")

    with tile.TileContext(nc) as tc_ctx, ExitStack() as ctx:
        pools = (
            ctx.enter_context(tc_ctx.tile_pool(name="mpool", bufs=MBUFS)),
            ctx.enter_context(tc_ctx.tile_pool(name="spool", bufs=SBUFS)),
            ctx.enter_context(tc_ctx.tile_pool(name="opool",
                                             bufs=OBUFS or max(3, 24 // ogroup))),
            ctx.enter_context(tc_ctx.tile_pool(name="ppool",
                                               bufs=max(1, 8 // PBANKS),
                                               space="PSUM")),
        )
        if reps == 1:
            _emit_body(tc_ctx, nc, x_d, m_d, o_d, pools, ogroup)
        else:
            with tc_ctx.For_i(0, reps, 1,
                              hint_engines=(mybir.EngineType.PE,
                                            mybir.EngineType.SP),
                              staggered_reset=bool(STAGGER)):
                _emit_body(tc_ctx, nc, x_d, m_d, o_d, pools, ogroup)

    nc.compile()
    return nc


_NC_CACHE = {}


def _get_module(reps=1, ogroup=None):
    key = (reps, ogroup)
    if key not in _NC_CACHE:
        _NC_CACHE[key] = build_module(reps, ogroup)
    return _NC_CACHE[key]


# ------------------------------------------------------------------ entrypoint
def run(x, reps=1, ogroup=None):
    """x: [16, 2048, 512, 1] float32 -> [16, 512, 2048, 4] float32."""
    nc = _get_module(reps, ogroup)
    x3 = np.asarray(x)[:, :, :, 0].astype(ml_dtypes.bfloat16)  # [B, T, N]
    # shifted k grid: block j, partition p holds k = 128j - SHIFT + p
    xp = np.zeros((B, NBLK * 128, N), dtype=ml_dtypes.bfloat16)
    xp[:, SHIFT:SHIFT + T, :] = x3
    # pre-tile to the SBUF layout: [b, nb_group, p, (blk j n)]
    sigw = 2 if SIGPAIR else 1
    xt = np.ascontiguousarray(
        xp.reshape(B, NBLK, 128, N // (128 * sigw), sigw, 128)
        .transpose(0, 3, 2, 4, 1, 5)
        .reshape(B, N // (128 * sigw), 128, sigw * NBLK * 128))
    in_maps = [
        {"x": xt[c * B_PER_CORE:(c + 1) * B_PER_CORE],
         **{f"m{s}": sd[5] for s, sd in enumerate(_M_SUBDMAS)}}
        for c in range(NCORES)
    ]
    res = run_bass_kernel_spmd(nc, in_maps, core_ids=list(range(NCORES)))
    out = np.concatenate([res.results[c]["out"] for c in range(NCORES)], axis=0)
    return out.astype(np.float32)


def kernel(x):
    return run(x)

